# revision 33
# baseline (speedup 1.0000x reference)
"""DigitCaps (CapsNet dynamic routing) Trainium2 Bass kernel.

Full computation per batch element b:
    u_hat[r,c,o] = sum_i u[r,i] * W[r,c,i,o]            (einsum)
    b_log = 0; for 3 iters: coef = softmax_c(b_log); s = sum_r coef*u_hat
                v = squash(s); b_log += sum_o u_hat*v
Output: v from last iteration.  Identity used: b_log(t) = u_hat . Vcum(t)
where Vcum = sum of previous v's, so logits are recomputed from Vcum
each iteration instead of accumulated.  Additionally softmax is kept
unnormalized: coef = exp(z), s~ = sum_r exp(z) u_hat, den = sum_r exp(z),
s = s~/den (softmax shift-invariance + late normalization).

Sharding: data-parallel over batch, 512 -> 8 cores x 64.

Per-core layout (P = 128 partitions):
  - einsum operands live in a "spread" layout: r = 16k + m (k in 0..72,
    m in 0..16); partition p(m,i) = (m%4)*32 + (m//4)*8 + i.  The 4
    MMs of 4 consecutive m hit 4 distinct 32-row groups of the PE
    array and run concurrently (tile_position row packing).
  - u_hat is resident in SBUF as bf16 [128 = (rq, b32); 288, 16(o), 10(c)]
    with partition = rq*32 + b_local (rq = r quartile), per 32-batch
    tile (2 tiles per core).  Free order is (rs, o, c) -- c innermost --
    so every elementwise product keeps a packed 2-byte last dim and runs
    in the DVE 2x perf mode, including broadcasts of V (over rs) and of
    E=exp(z) (over o).
  - iter-0 s (uniform coef 0.1) is a clean K=128 chunked matmul.
  - routing iters 1..2: products u_hat*V / u_hat*E split DVE/GPSIMD;
    all o/rs reductions are pairwise tree-adds in bf16 (2x mode) instead
    of TensorReduce (which has no DVE perf modes).  Cross-partition-group
    reductions (sum over the 4 rq groups) and the V broadcast to the
    (rq,b) layout are done with tiny constant matmuls (REP/REPT).
"""

import sys

sys.path.insert(0, "/opt/trn_rl_repo")

import functools
from contextlib import ExitStack

import numpy as np

NCORES = 8
B = 64          # batch per core
BT = 32         # batch tile
R = 1152
C = 10
I = 8
O = 16
CO = C * O      # 160
NK = 72         # r-chunks of 16
RQ_K = 18       # k's per r-quartile (288 r's)
RL = 288        # r_loc per quartile
ZCH = 32        # r_locs per routing chunk
NCH = RL // ZCH  # 9
POOL_PROD = 10   # of the 18 products per tile-iter, how many go to Pool
USE_TPOS = True     # pass explicit tile_position on einsum MMs


def _wslice(w):
    return slice(w * 32, (w + 1) * 32)


def build_bass(phase: str = "full"):
    import concourse.bass as bass
    import concourse.tile as tile
    from concourse import bacc, mybir
    from concourse.masks import make_identity

    f32 = mybir.dt.float32
    bf16 = mybir.dt.bfloat16
    AX = mybir.AxisListType
    OP = mybir.AluOpType
    AF = mybir.ActivationFunctionType

    nc = bacc.Bacc(
        "TRN2",
        target_bir_lowering=False,
        debug=False,
        enable_asserts=False,
        num_devices=NCORES,
    )
    # u and W arrive pre-arranged (host side) in the SBUF spread layouts:
    # ut[p, k, b] = u[b, 16k+m(p), i(p)], wsb[p, k, o, c] = W[16k+m(p), c,
    # i(p), o] with p(m,i) = (m%4)*32 + (m//4)*8 + i, both bf16 -> prep is
    # two big contiguous DMAs instead of 160 strided gathers + casts.
    u_d = nc.dram_tensor("ut", [128, NK, B], bf16, kind="ExternalInput").ap()
    w_d = nc.dram_tensor("wsb", [128, NK, O, C], bf16, kind="ExternalInput").ap()
    v_d = nc.dram_tensor("v", [B, C, O], f32, kind="ExternalOutput").ap()
    m_d = nc.dram_tensor("msk", [128, 4], f32, kind="ExternalInput").ap()

    with tile.TileContext(nc) as tc, ExitStack() as ctx:
        # ---------------- persistent pools ----------------
        consts = ctx.enter_context(tc.tile_pool(name="consts", bufs=1))
        persist = ctx.enter_context(tc.tile_pool(name="persist", bufs=1))

        # uTz[j] holds u^T (partition p = (r%16)*8 + i) with only the
        # rows of m % 4 == j kept, zeros elsewhere -> a K=32 matmul on a
        # 32-aligned window isolates one r.
        uTz0 = persist.tile([128, NK, B], bf16)
        uTz1 = persist.tile([128, NK, B], bf16)
        uTz2 = persist.tile([128, NK, B], bf16)
        uTz3 = persist.tile([128, NK, B], bf16)
        uTz = [uTz0, uTz1, uTz2, uTz3]
        W_sb = persist.tile([128, NK, O, C], bf16)  # 23 KB/part, (o,c) order

        def ecopy(which, out_ap, in_ap):
            if which == 0:
                nc.vector.tensor_copy(out_ap, in_ap)
            else:
                nc.scalar.copy(out_ap, in_ap)

        id64 = consts.tile([64, 64], bf16)
        make_identity(nc, id64)
        id32 = consts.tile([32, 32], f32)
        make_identity(nc, id32)

        # REP[b, (q, b')] = 1.0 iff b == b'   (bf16, [32, 4, 32])
        REPf = consts.tile([32, 4, 32], f32)
        nc.gpsimd.memset(REPf[:], 0.0)
        nc.gpsimd.affine_select(
            out=REPf[:],
            in_=REPf[:],
            compare_op=OP.not_equal,
            fill=1.0,
            base=0,
            pattern=[[0, 4], [-1, 32]],
            channel_multiplier=1,
        )
        REPT = consts.tile([128, 32], bf16)

        # ---------------- routing-side pools needed inside prep (iter0) --
        rt = ctx.enter_context(tc.tile_pool(name="rt", bufs=1))
        sm = ctx.enter_context(tc.tile_pool(name="sm", bufs=1))
        logits = rt.tile([128, RL, C], f32)          # 11.5 KB
        E_slim = rt.tile([128, RL, C], bf16)         # 5.6 KB
        den = rt.tile([128, RL], f32)
        V_rep = rt.tile([128, O, C], bf16)
        s_acc = rt.tile([128, O, C], f32)
        s_bf = rt.tile([128, CO], bf16)
        v0 = rt.tile([64, O, C], f32)
        Vcb0 = rt.tile([32, O, C], f32)
        Vcb1 = rt.tile([32, O, C], f32)
        Vcb = [Vcb0, Vcb1]

        def squash(p, s_ap, out_ap, pool):
            # out = |s| / (1 + |s|^2) * s   per (partition, c); s is [p, O, C]
            sq = pool.tile([p, O, C], f32, tag="sqt")
            nc.vector.tensor_mul(sq[:], s_ap, s_ap)
            ssum = pool.tile([p, C], f32, tag="sst")
            nc.vector.tensor_reduce(
                ssum[:], sq[:].rearrange("p o c -> p c o"), axis=AX.X, op=OP.add
            )
            norm = pool.tile([p, C], f32, tag="snt")
            nc.scalar.sqrt(norm[:], ssum[:])
            onep = pool.tile([p, C], f32, tag="sot")
            nc.scalar.add(onep[:], ssum[:], 1.0)
            rec = pool.tile([p, C], f32, tag="srt")
            nc.vector.reciprocal(rec[:], onep[:])
            fac = pool.tile([p, C], f32, tag="sft")
            nc.vector.tensor_mul(fac[:], norm[:], rec[:])
            nc.vector.tensor_mul(
                out_ap,
                s_ap,
                fac[:].unsqueeze(1).broadcast_to((p, O, C)),
            )

        def iter0(s0ps):
            s_all = rt.tile([64, O, C], f32)
            nc.scalar.mul(
                s_all[:], s0ps[:].rearrange("p (o c) -> p o c", o=O), 0.1
            )
            squash(64, s_all[:], v0[:], sm)
            nc.vector.tensor_copy(Vcb[0][:], v0[0:32, :, :])
            nc.sync.dma_start(out=Vcb[1][:], in_=v0[32:64, :, :])

        # ---------------- prep phase ----------------
        with ExitStack() as prep:
            pp = prep.enter_context(tc.tile_pool(name="prep", bufs=1))
            wch = prep.enter_context(tc.tile_pool(name="wch", bufs=1))
            ppsum = prep.enter_context(
                tc.tile_pool(name="ppsum", bufs=2, space="PSUM")
            )
            s0_pool = prep.enter_context(
                tc.tile_pool(name="s0psp", bufs=1, space="PSUM")
            )
            s0ps = s0_pool.tile([64, CO], f32)

            # REPT = REP^T via PE
            rps = ppsum.tile([128, 32], f32)
            nc.tensor.transpose(
                rps[:], REPf[:].rearrange("b q c -> b (q c)"), id32[:]
            )
            nc.vector.tensor_copy(REPT[:], rps[:])

            # u^T and W: single contiguous DMAs into the spread layouts
            uT_full = pp.tile([128, NK, B], bf16)
            nc.sync.dma_start(
                out=uT_full[:].rearrange("p k b -> p (k b)"),
                in_=u_d.rearrange("p k b -> p (k b)"),
            )
            nc.gpsimd.dma_start(
                out=W_sb[:].rearrange("p k o c -> p (k o c)"),
                in_=w_d.rearrange("p k o c -> p (k o c)"),
            )
            msk = pp.tile([128, 4], f32)
            nc.sync.dma_start(out=msk[:], in_=m_d)
            for j in range(4):
                nc.vector.tensor_scalar_mul(
                    uTz[j][:].rearrange("p k b -> p (k b)"),
                    uT_full[:].rearrange("p k b -> p (k b)"),
                    msk[:, j : j + 1],
                )

            # iter-0 s matmul chain while the full u^T is still alive:
            # s0 = sum_k uT_full[:,k,:].T @ W_sb[:,k,:]  (all 64 b at once)
            for k in range(NK):
                nc.tensor.matmul(
                    s0ps[:],
                    uT_full[:, k, :],
                    W_sb[:, k, :, :],
                    start=(k == 0),
                    stop=(k == NK - 1),
                )
            if phase != "prep":
                iter0(s0ps)

        # ---------------- main pools ----------------
        big = ctx.enter_context(tc.tile_pool(name="big", bufs=1))
        scratch = ctx.enter_context(tc.tile_pool(name="scratch", bufs=2))
        u_hat = big.tile([128, RL, O, C], bf16)      # 92 KB/part, (rs,o,c)

        # ---------------- einsum: u_hat per batch tile ----------------
        def einsum_tile(bt, mm_psum):
            # storage index within a 16-block: rs = j*4 + w (the 4
            # w-concurrent MMs land on consecutive r slots).  Any r
            # permutation is fine: routing is symmetric in r.
            for kl in range(RQ_K):
                for j in range(4):
                    pe_ps = mm_psum.tile([128, 4, 512], f32, tag="pe")
                    for rq in range(4):
                        k = rq * RQ_K + kl
                        for w in range(4):
                            nc.tensor.matmul(
                                pe_ps[rq * 32 : (rq + 1) * 32, w, 0:CO],
                                uTz[j][_wslice(w), k, bt * BT : (bt + 1) * BT],
                                W_sb[_wslice(w), k, :, :],
                                start=True,
                                stop=True,
                                tile_position=(w * 32, rq * 32),
                            )
                    rs0 = 16 * kl + j * 4
                    dst = u_hat[:, rs0 : rs0 + 4, :, :].rearrange(
                        "p r o c -> p r (o c)"
                    )
                    ecopy(j % 2, dst, pe_ps[:, :, 0:CO])

        # ---------------- routing iteration ----------------
        def routing_iter(bt, t, it_psum):
            # V broadcast to (rq, b) layout:  V_rep = REP^T-ish matmul
            vps = it_psum.tile([128, CO], f32, tag="vrep")
            nc.tensor.matmul(
                vps[:],
                REPf[:].rearrange("b q c -> b (q c)"),
                Vcb[bt][:].rearrange("p o c -> p (o c)"),
                start=True,
                stop=True,
            )
            nc.vector.tensor_copy(
                V_rep[:].rearrange("p o c -> p (o c)"), vps[:]
            )
            # z-pass: logits = sum_o u_hat * V.  V_rep broadcasts over rs
            # (non-last stride-0, packed c last -> still 2x).  Reduction
            # over o via pairwise tree adds in bf16 (2x), final level f32.
            for rc in range(NCH):
                uh = u_hat[:, rc * ZCH : (rc + 1) * ZCH, :, :].rearrange(
                    "p r o c -> p r (o c)"
                )
                pr = scratch.tile([128, ZCH, CO], bf16, tag="pr")
                vb = (
                    V_rep[:].rearrange("p o c -> p (o c)")
                    .unsqueeze(1).broadcast_to((128, ZCH, CO))
                )
                # Pool chunks start at rc=2 so DVE's in-order queue is never
                # head-of-line blocked at pass start waiting on the slower
                # Pool product.
                peng = nc.gpsimd if rc % 3 == 2 else nc.vector
                peng.tensor_mul(pr[:], uh, vb)
                # in-place pairwise tree over o: halves of the contiguous
                # (o,c) block stay contiguous slices (bf16 2x mode)
                nc.vector.tensor_add(pr[:, :, 0:80], pr[:, :, 0:80], pr[:, :, 80:160])
                nc.vector.tensor_add(pr[:, :, 0:40], pr[:, :, 0:40], pr[:, :, 40:80])
                nc.vector.tensor_add(pr[:, :, 0:20], pr[:, :, 0:20], pr[:, :, 20:40])
                nc.vector.tensor_add(
                    logits[:, rc * ZCH : (rc + 1) * ZCH, :],
                    pr[:, :, 0:10], pr[:, :, 10:20],
                )
            # softmax over c: E = exp(logits) then normalize by the per-r
            # class sum.  No max-subtraction: |logits| <~ 60 is inside fp32
            # exp range.
            nc.scalar.activation(
                logits[:].rearrange("p r c -> p (r c)"),
                logits[:].rearrange("p r c -> p (r c)"),
                AF.Exp,
            )
            nc.vector.tensor_reduce(den[:], logits[:], axis=AX.X, op=OP.add)
            nc.vector.reciprocal(den[:], den[:])
            nc.vector.tensor_mul(
                E_slim[:],
                logits[:],
                den[:].unsqueeze(2).broadcast_to((128, RL, C)),
            )
            # s-pass: s_acc = sum_rloc E * u_hat (E broadcast over o; packed
            # c last -> 2x).  rs reduced by tree adds (halving the rs dim).
            nc.gpsimd.memset(s_acc[:], 0.0)
            for rc in range(NCH):
                uh4 = u_hat[:, rc * ZCH : (rc + 1) * ZCH, :, :]
                pr2 = scratch.tile([128, ZCH, O, C], bf16, tag="pr")
                peng = nc.gpsimd if rc % 3 == 1 else nc.vector
                # E broadcast over o is inherently 4D -> TensorTensor (2x)
                peng.tensor_mul(
                    pr2[:],
                    uh4,
                    E_slim[:, rc * ZCH : (rc + 1) * ZCH, :]
                    .unsqueeze(2)
                    .broadcast_to((128, ZCH, O, C)),
                )
                # in-place pairwise tree over rs (outer dim, bf16 2x)
                p3 = pr2[:].rearrange("p r o c -> p r (o c)")
                nc.vector.tensor_add(p3[:, 0:16], p3[:, 0:16], p3[:, 16:32])
                nc.vector.tensor_add(p3[:, 0:8], p3[:, 0:8], p3[:, 8:16])
                nc.vector.tensor_add(p3[:, 0:4], p3[:, 0:4], p3[:, 4:8])
                nc.vector.tensor_add(p3[:, 0:2], p3[:, 0:2], p3[:, 2:4])
                nc.vector.tensor_add(p3[:, 0], p3[:, 0], p3[:, 1])
                nc.vector.tensor_add(
                    s_acc[:].rearrange("p o c -> p (o c)"),
                    s_acc[:].rearrange("p o c -> p (o c)"), p3[:, 0]
                )
            # combine the 4 rq groups via one REPT matmul
            nc.vector.tensor_copy(
                s_bf[:], s_acc[:].rearrange("p o c -> p (o c)")
            )
            scps = it_psum.tile([32, CO], f32, tag="comb")
            nc.tensor.matmul(
                scps[:], REPT[:], s_bf[:], start=True, stop=True
            )
            s_bt = sm.tile([32, O, C], f32, tag="sbt")
            nc.vector.tensor_copy(
                s_bt[:].rearrange("p o c -> p (o c)"), scps[:]
            )
            if t == 1:
                v_t = sm.tile([32, O, C], f32, tag="vt")
                squash(32, s_bt[:], v_t[:], sm)
                nc.vector.tensor_add(Vcb[bt][:], Vcb[bt][:], v_t[:])
            else:
                # write v in (c, o) order so the output DMA is contiguous
                v_out = sm.tile([32, C, O], f32, tag="vout")
                squash(32, s_bt[:], v_out[:].rearrange("p c o -> p o c"), sm)
                nc.sync.dma_start(
                    out=v_d[bt * BT : (bt + 1) * BT, :, :], in_=v_out[:]
                )

        if phase == "prep":
            pass
        elif phase == "einsum":
            with ExitStack() as es:
                mm_psum = es.enter_context(
                    tc.tile_pool(name="mmps0", bufs=2, space="PSUM")
                )
                einsum_tile(0, mm_psum)
        else:
            for bt in range(2):
                with ExitStack() as es:
                    mm_psum = es.enter_context(
                        tc.tile_pool(name=f"mmps{bt}", bufs=2, space="PSUM")
                    )
                    einsum_tile(bt, mm_psum)
                with ExitStack() as es:
                    it_psum = es.enter_context(
                        tc.tile_pool(name=f"itps{bt}", bufs=2, space="PSUM")
                    )
                    for t in (1, 2):
                        routing_iter(bt, t, it_psum)

    nc.compile()
    return nc


@functools.cache
def _get_nc():
    return build_bass()


def make_mask() -> np.ndarray:
    p = np.arange(128)
    j = (p // I) % 4
    return (j[:, None] == np.arange(4)[None, :]).astype(np.float32)


def kernel(u: np.ndarray, W: np.ndarray) -> np.ndarray:
    import ml_dtypes
    from concourse import bass_utils

    nc = _get_nc()
    bf = ml_dtypes.bfloat16
    W4 = np.ascontiguousarray(W.reshape(R, C, I, O)).astype(np.float32)
    # spread layout: partition p(m, i) = m*8 + i (transpose column order)
    p = np.arange(128)
    m = p // 8
    i = p % 8
    r_idx = 16 * np.arange(NK)[None, :] + m[:, None]          # [128, NK]
    # wsb[p, k, o, c] = W[16k+m(p), c, i(p), o]
    wsb = np.ascontiguousarray(
        W4[r_idx, :, i[:, None], :].transpose(0, 1, 3, 2)
    ).astype(bf)
    msk = make_mask()
    in_maps = []
    for core in range(NCORES):
        uc = np.asarray(u[core * B : (core + 1) * B], dtype=np.float32)
        # ut[p, k, b] = u[b, 16k+m(p), i(p)]
        ut = np.ascontiguousarray(
            uc[:, r_idx, i[:, None]].transpose(1, 2, 0)
        ).astype(bf)
        in_maps.append({"ut": ut, "wsb": wsb, "msk": msk})
    res = bass_utils.run_bass_kernel_spmd(
        nc, in_maps, core_ids=list(range(NCORES))
    )
    return np.concatenate([r["v"] for r in res.results], axis=0)


# revision 34
# speedup vs baseline: 1.0005x; 1.0005x over previous
"""DigitCaps (CapsNet dynamic routing) Trainium2 Bass kernel.

Full computation per batch element b:
    u_hat[r,c,o] = sum_i u[r,i] * W[r,c,i,o]            (einsum)
    b_log = 0; for 3 iters: coef = softmax_c(b_log); s = sum_r coef*u_hat
                v = squash(s); b_log += sum_o u_hat*v
Output: v from last iteration.  Identity used: b_log(t) = u_hat . Vcum(t)
where Vcum = sum of previous v's, so logits are recomputed from Vcum
each iteration instead of accumulated.  Additionally softmax is kept
unnormalized: coef = exp(z), s~ = sum_r exp(z) u_hat, den = sum_r exp(z),
s = s~/den (softmax shift-invariance + late normalization).

Sharding: data-parallel over batch, 512 -> 8 cores x 64.

Per-core layout (P = 128 partitions):
  - einsum operands live in a "spread" layout: r = 16k + m (k in 0..72,
    m in 0..16); partition p(m,i) = (m%4)*32 + (m//4)*8 + i.  The 4
    MMs of 4 consecutive m hit 4 distinct 32-row groups of the PE
    array and run concurrently (tile_position row packing).
  - u_hat is resident in SBUF as bf16 [128 = (rq, b32); 288, 16(o), 10(c)]
    with partition = rq*32 + b_local (rq = r quartile), per 32-batch
    tile (2 tiles per core).  Free order is (rs, o, c) -- c innermost --
    so every elementwise product keeps a packed 2-byte last dim and runs
    in the DVE 2x perf mode, including broadcasts of V (over rs) and of
    E=exp(z) (over o).
  - iter-0 s (uniform coef 0.1) is a clean K=128 chunked matmul.
  - routing iters 1..2: products u_hat*V / u_hat*E split DVE/GPSIMD;
    all o/rs reductions are pairwise tree-adds in bf16 (2x mode) instead
    of TensorReduce (which has no DVE perf modes).  Cross-partition-group
    reductions (sum over the 4 rq groups) and the V broadcast to the
    (rq,b) layout are done with tiny constant matmuls (REP/REPT).
"""

import sys

sys.path.insert(0, "/opt/trn_rl_repo")

import functools
from contextlib import ExitStack

import numpy as np

NCORES = 8
B = 64          # batch per core
BT = 32         # batch tile
R = 1152
C = 10
I = 8
O = 16
CO = C * O      # 160
NK = 72         # r-chunks of 16
RQ_K = 18       # k's per r-quartile (288 r's)
RL = 288        # r_loc per quartile
ZCH = 32        # r_locs per routing chunk
NCH = RL // ZCH  # 9
POOL_PROD = 10   # of the 18 products per tile-iter, how many go to Pool
USE_TPOS = True     # pass explicit tile_position on einsum MMs


def _wslice(w):
    return slice(w * 32, (w + 1) * 32)


def build_bass(phase: str = "full"):
    import concourse.bass as bass
    import concourse.tile as tile
    from concourse import bacc, mybir
    from concourse.masks import make_identity

    f32 = mybir.dt.float32
    bf16 = mybir.dt.bfloat16
    AX = mybir.AxisListType
    OP = mybir.AluOpType
    AF = mybir.ActivationFunctionType

    nc = bacc.Bacc(
        "TRN2",
        target_bir_lowering=False,
        debug=False,
        enable_asserts=False,
        num_devices=NCORES,
    )
    # u and W arrive pre-arranged (host side) in the SBUF spread layouts:
    # ut[p, k, b] = u[b, 16k+m(p), i(p)], wsb[p, k, o, c] = W[16k+m(p), c,
    # i(p), o] with p(m,i) = (m%4)*32 + (m//4)*8 + i, both bf16 -> prep is
    # two big contiguous DMAs instead of 160 strided gathers + casts.
    u_d = nc.dram_tensor("ut", [128, NK, B], bf16, kind="ExternalInput").ap()
    w_d = nc.dram_tensor("wsb", [128, NK, O, C], bf16, kind="ExternalInput").ap()
    v_d = nc.dram_tensor("v", [B, C, O], f32, kind="ExternalOutput").ap()
    m_d = nc.dram_tensor("msk", [128, 4], f32, kind="ExternalInput").ap()

    with tile.TileContext(nc) as tc, ExitStack() as ctx:
        # ---------------- persistent pools ----------------
        consts = ctx.enter_context(tc.tile_pool(name="consts", bufs=1))
        persist = ctx.enter_context(tc.tile_pool(name="persist", bufs=1))

        # uTz[j] holds u^T (partition p = (r%16)*8 + i) with only the
        # rows of m % 4 == j kept, zeros elsewhere -> a K=32 matmul on a
        # 32-aligned window isolates one r.
        uTz0 = persist.tile([128, NK, B], bf16)
        uTz1 = persist.tile([128, NK, B], bf16)
        uTz2 = persist.tile([128, NK, B], bf16)
        uTz3 = persist.tile([128, NK, B], bf16)
        uTz = [uTz0, uTz1, uTz2, uTz3]
        W_sb = persist.tile([128, NK, O, C], bf16)  # 23 KB/part, (o,c) order

        def ecopy(which, out_ap, in_ap):
            if which == 0:
                nc.vector.tensor_copy(out_ap, in_ap)
            else:
                nc.scalar.copy(out_ap, in_ap)

        id64 = consts.tile([64, 64], bf16)
        make_identity(nc, id64)
        id32 = consts.tile([32, 32], f32)
        make_identity(nc, id32)

        # REP[b, (q, b')] = 1.0 iff b == b'   (bf16, [32, 4, 32])
        REPf = consts.tile([32, 4, 32], f32)
        nc.gpsimd.memset(REPf[:], 0.0)
        nc.gpsimd.affine_select(
            out=REPf[:],
            in_=REPf[:],
            compare_op=OP.not_equal,
            fill=1.0,
            base=0,
            pattern=[[0, 4], [-1, 32]],
            channel_multiplier=1,
        )
        REPT = consts.tile([128, 32], bf16)

        # ---------------- routing-side pools needed inside prep (iter0) --
        rt = ctx.enter_context(tc.tile_pool(name="rt", bufs=1))
        sm = ctx.enter_context(tc.tile_pool(name="sm", bufs=1))
        logits = rt.tile([128, RL, C], f32)          # 11.5 KB
        E_slim = rt.tile([128, RL, C], bf16)         # 5.6 KB
        den = rt.tile([128, RL], f32)
        V_rep = rt.tile([128, O, C], bf16)
        s_acc = rt.tile([128, O, C], f32)
        s_bf = rt.tile([128, CO], bf16)
        v0 = rt.tile([64, O, C], f32)
        Vcb0 = rt.tile([32, O, C], f32)
        Vcb1 = rt.tile([32, O, C], f32)
        Vcb = [Vcb0, Vcb1]

        def squash(p, s_ap, out_ap, pool):
            # out = |s| / (1 + |s|^2) * s   per (partition, c); s is [p, O, C]
            sq = pool.tile([p, O, C], f32, tag="sqt")
            nc.vector.tensor_mul(sq[:], s_ap, s_ap)
            ssum = pool.tile([p, C], f32, tag="sst")
            nc.vector.tensor_reduce(
                ssum[:], sq[:].rearrange("p o c -> p c o"), axis=AX.X, op=OP.add
            )
            norm = pool.tile([p, C], f32, tag="snt")
            nc.scalar.sqrt(norm[:], ssum[:])
            onep = pool.tile([p, C], f32, tag="sot")
            nc.scalar.add(onep[:], ssum[:], 1.0)
            rec = pool.tile([p, C], f32, tag="srt")
            nc.vector.reciprocal(rec[:], onep[:])
            fac = pool.tile([p, C], f32, tag="sft")
            nc.vector.tensor_mul(fac[:], norm[:], rec[:])
            nc.vector.tensor_mul(
                out_ap,
                s_ap,
                fac[:].unsqueeze(1).broadcast_to((p, O, C)),
            )

        def iter0(s0ps):
            s_all = rt.tile([64, O, C], f32)
            nc.scalar.mul(
                s_all[:], s0ps[:].rearrange("p (o c) -> p o c", o=O), 0.1
            )
            squash(64, s_all[:], v0[:], sm)
            nc.vector.tensor_copy(Vcb[0][:], v0[0:32, :, :])
            nc.sync.dma_start(out=Vcb[1][:], in_=v0[32:64, :, :])

        # ---------------- prep phase ----------------
        with ExitStack() as prep:
            pp = prep.enter_context(tc.tile_pool(name="prep", bufs=1))
            wch = prep.enter_context(tc.tile_pool(name="wch", bufs=1))
            ppsum = prep.enter_context(
                tc.tile_pool(name="ppsum", bufs=2, space="PSUM")
            )
            s0_pool = prep.enter_context(
                tc.tile_pool(name="s0psp", bufs=1, space="PSUM")
            )
            s0ps = s0_pool.tile([64, CO], f32)

            # REPT = REP^T via PE
            rps = ppsum.tile([128, 32], f32)
            nc.tensor.transpose(
                rps[:], REPf[:].rearrange("b q c -> b (q c)"), id32[:]
            )
            nc.vector.tensor_copy(REPT[:], rps[:])

            # u^T and W: single contiguous DMAs into the spread layouts
            uT_full = pp.tile([128, NK, B], bf16)
            nc.sync.dma_start(
                out=uT_full[:].rearrange("p k b -> p (k b)"),
                in_=u_d.rearrange("p k b -> p (k b)"),
            )
            nc.gpsimd.dma_start(
                out=W_sb[:].rearrange("p k o c -> p (k o c)"),
                in_=w_d.rearrange("p k o c -> p (k o c)"),
            )
            msk = pp.tile([128, 4], f32)
            nc.sync.dma_start(out=msk[:], in_=m_d)
            for j in range(4):
                nc.vector.tensor_scalar_mul(
                    uTz[j][:].rearrange("p k b -> p (k b)"),
                    uT_full[:].rearrange("p k b -> p (k b)"),
                    msk[:, j : j + 1],
                )

            # iter-0 s matmul chain while the full u^T is still alive:
            # s0 = sum_k uT_full[:,k,:].T @ W_sb[:,k,:]  (all 64 b at once)
            for k in range(NK):
                nc.tensor.matmul(
                    s0ps[:],
                    uT_full[:, k, :],
                    W_sb[:, k, :, :],
                    start=(k == 0),
                    stop=(k == NK - 1),
                )
            if phase != "prep":
                iter0(s0ps)

        # ---------------- main pools ----------------
        big = ctx.enter_context(tc.tile_pool(name="big", bufs=1))
        scratch = ctx.enter_context(tc.tile_pool(name="scratch", bufs=2))
        u_hat = big.tile([128, RL, O, C], bf16)      # 92 KB/part, (rs,o,c)

        # ---------------- einsum: u_hat per batch tile ----------------
        def einsum_tile(bt, mm_psum):
            # storage index within a 16-block: rs = j*4 + w (the 4
            # w-concurrent MMs land on consecutive r slots).  Any r
            # permutation is fine: routing is symmetric in r.
            for kl in range(RQ_K):
                for j in range(4):
                    pe_ps = mm_psum.tile([128, 4, 512], f32, tag="pe")
                    for rq in range(4):
                        k = rq * RQ_K + kl
                        for w in range(4):
                            nc.tensor.matmul(
                                pe_ps[rq * 32 : (rq + 1) * 32, w, 0:CO],
                                uTz[j][_wslice(w), k, bt * BT : (bt + 1) * BT],
                                W_sb[_wslice(w), k, :, :],
                                start=True,
                                stop=True,
                                tile_position=(w * 32, rq * 32),
                            )
                    rs0 = 16 * kl + j * 4
                    dst = u_hat[:, rs0 : rs0 + 4, :, :].rearrange(
                        "p r o c -> p r (o c)"
                    )
                    ecopy(j % 2, dst, pe_ps[:, :, 0:CO])

        # ---------------- routing iteration ----------------
        def routing_iter(bt, t, it_psum):
            # V broadcast to (rq, b) layout:  V_rep = REP^T-ish matmul
            vps = it_psum.tile([128, CO], f32, tag="vrep")
            nc.tensor.matmul(
                vps[:],
                REPf[:].rearrange("b q c -> b (q c)"),
                Vcb[bt][:].rearrange("p o c -> p (o c)"),
                start=True,
                stop=True,
            )
            nc.vector.tensor_copy(
                V_rep[:].rearrange("p o c -> p (o c)"), vps[:]
            )
            # z-pass: logits = sum_o u_hat * V.  V_rep broadcasts over rs
            # (non-last stride-0, packed c last -> still 2x).  Reduction
            # over o via pairwise tree adds in bf16 (2x), final level f32.
            for rc in range(NCH):
                uh = u_hat[:, rc * ZCH : (rc + 1) * ZCH, :, :].rearrange(
                    "p r o c -> p r (o c)"
                )
                pr = scratch.tile([128, ZCH, CO], bf16, tag="pr")
                vb = (
                    V_rep[:].rearrange("p o c -> p (o c)")
                    .unsqueeze(1).broadcast_to((128, ZCH, CO))
                )
                peng = nc.gpsimd if rc % 4 == 0 else nc.vector
                peng.tensor_mul(pr[:], uh, vb)
                # in-place pairwise tree over o: halves of the contiguous
                # (o,c) block stay contiguous slices (bf16 2x mode)
                nc.vector.tensor_add(pr[:, :, 0:80], pr[:, :, 0:80], pr[:, :, 80:160])
                nc.vector.tensor_add(pr[:, :, 0:40], pr[:, :, 0:40], pr[:, :, 40:80])
                nc.vector.tensor_add(pr[:, :, 0:20], pr[:, :, 0:20], pr[:, :, 20:40])
                nc.vector.tensor_add(
                    logits[:, rc * ZCH : (rc + 1) * ZCH, :],
                    pr[:, :, 0:10], pr[:, :, 10:20],
                )
            # softmax over c: E = exp(logits) then normalize by the per-r
            # class sum.  No max-subtraction: |logits| <~ 60 is inside fp32
            # exp range.
            nc.scalar.activation(
                logits[:].rearrange("p r c -> p (r c)"),
                logits[:].rearrange("p r c -> p (r c)"),
                AF.Exp,
            )
            nc.vector.tensor_reduce(den[:], logits[:], axis=AX.X, op=OP.add)
            nc.vector.reciprocal(den[:], den[:])
            nc.vector.tensor_mul(
                E_slim[:],
                logits[:],
                den[:].unsqueeze(2).broadcast_to((128, RL, C)),
            )
            # s-pass: s_acc = sum_rloc E * u_hat (E broadcast over o; packed
            # c last -> 2x).  rs reduced by tree adds (halving the rs dim).
            nc.gpsimd.memset(s_acc[:], 0.0)
            for rc in range(NCH):
                uh4 = u_hat[:, rc * ZCH : (rc + 1) * ZCH, :, :]
                pr2 = scratch.tile([128, ZCH, O, C], bf16, tag="pr")
                peng = nc.gpsimd if rc % 3 == 1 else nc.vector
                # E broadcast over o is inherently 4D -> TensorTensor (2x)
                peng.tensor_mul(
                    pr2[:],
                    uh4,
                    E_slim[:, rc * ZCH : (rc + 1) * ZCH, :]
                    .unsqueeze(2)
                    .broadcast_to((128, ZCH, O, C)),
                )
                # in-place pairwise tree over rs (outer dim, bf16 2x)
                p3 = pr2[:].rearrange("p r o c -> p r (o c)")
                nc.vector.tensor_add(p3[:, 0:16], p3[:, 0:16], p3[:, 16:32])
                nc.vector.tensor_add(p3[:, 0:8], p3[:, 0:8], p3[:, 8:16])
                nc.vector.tensor_add(p3[:, 0:4], p3[:, 0:4], p3[:, 4:8])
                nc.vector.tensor_add(p3[:, 0:2], p3[:, 0:2], p3[:, 2:4])
                nc.vector.tensor_add(p3[:, 0], p3[:, 0], p3[:, 1])
                nc.vector.tensor_add(
                    s_acc[:].rearrange("p o c -> p (o c)"),
                    s_acc[:].rearrange("p o c -> p (o c)"), p3[:, 0]
                )
            # combine the 4 rq groups via one REPT matmul
            nc.vector.tensor_copy(
                s_bf[:], s_acc[:].rearrange("p o c -> p (o c)")
            )
            scps = it_psum.tile([32, CO], f32, tag="comb")
            nc.tensor.matmul(
                scps[:], REPT[:], s_bf[:], start=True, stop=True
            )
            s_bt = sm.tile([32, O, C], f32, tag="sbt")
            nc.vector.tensor_copy(
                s_bt[:].rearrange("p o c -> p (o c)"), scps[:]
            )
            if t == 1:
                v_t = sm.tile([32, O, C], f32, tag="vt")
                squash(32, s_bt[:], v_t[:], sm)
                nc.vector.tensor_add(Vcb[bt][:], Vcb[bt][:], v_t[:])
            else:
                # write v in (c, o) order so the output DMA is contiguous
                v_out = sm.tile([32, C, O], f32, tag="vout")
                squash(32, s_bt[:], v_out[:].rearrange("p c o -> p o c"), sm)
                nc.sync.dma_start(
                    out=v_d[bt * BT : (bt + 1) * BT, :, :], in_=v_out[:]
                )

        if phase == "prep":
            pass
        elif phase == "einsum":
            with ExitStack() as es:
                mm_psum = es.enter_context(
                    tc.tile_pool(name="mmps0", bufs=2, space="PSUM")
                )
                einsum_tile(0, mm_psum)
        else:
            for bt in range(2):
                with ExitStack() as es:
                    mm_psum = es.enter_context(
                        tc.tile_pool(name=f"mmps{bt}", bufs=2, space="PSUM")
                    )
                    einsum_tile(bt, mm_psum)
                with ExitStack() as es:
                    it_psum = es.enter_context(
                        tc.tile_pool(name=f"itps{bt}", bufs=2, space="PSUM")
                    )
                    for t in (1, 2):
                        routing_iter(bt, t, it_psum)

    nc.compile()
    return nc


@functools.cache
def _get_nc():
    return build_bass()


def make_mask() -> np.ndarray:
    p = np.arange(128)
    j = (p // I) % 4
    return (j[:, None] == np.arange(4)[None, :]).astype(np.float32)


def kernel(u: np.ndarray, W: np.ndarray) -> np.ndarray:
    import ml_dtypes
    from concourse import bass_utils

    nc = _get_nc()
    bf = ml_dtypes.bfloat16
    W4 = np.ascontiguousarray(W.reshape(R, C, I, O)).astype(np.float32)
    # spread layout: partition p(m, i) = m*8 + i (transpose column order)
    p = np.arange(128)
    m = p // 8
    i = p % 8
    r_idx = 16 * np.arange(NK)[None, :] + m[:, None]          # [128, NK]
    # wsb[p, k, o, c] = W[16k+m(p), c, i(p), o]
    wsb = np.ascontiguousarray(
        W4[r_idx, :, i[:, None], :].transpose(0, 1, 3, 2)
    ).astype(bf)
    msk = make_mask()
    in_maps = []
    for core in range(NCORES):
        uc = np.asarray(u[core * B : (core + 1) * B], dtype=np.float32)
        # ut[p, k, b] = u[b, 16k+m(p), i(p)]
        ut = np.ascontiguousarray(
            uc[:, r_idx, i[:, None]].transpose(1, 2, 0)
        ).astype(bf)
        in_maps.append({"ut": ut, "wsb": wsb, "msk": msk})
    res = bass_utils.run_bass_kernel_spmd(
        nc, in_maps, core_ids=list(range(NCORES))
    )
    return np.concatenate([r["v"] for r in res.results], axis=0)


# revision 38
# speedup vs baseline: 1.0454x; 1.0449x over previous
"""DigitCaps (CapsNet dynamic routing) Trainium2 Bass kernel.

Full computation per batch element b:
    u_hat[r,c,o] = sum_i u[r,i] * W[r,c,i,o]            (einsum)
    b_log = 0; for 3 iters: coef = softmax_c(b_log); s = sum_r coef*u_hat
                v = squash(s); b_log += sum_o u_hat*v
Output: v from last iteration.  Identity used: b_log(t) = u_hat . Vcum(t)
where Vcum = sum of previous v's, so logits are recomputed from Vcum
each iteration instead of accumulated.  Additionally softmax is kept
unnormalized: coef = exp(z), s~ = sum_r exp(z) u_hat, den = sum_r exp(z),
s = s~/den (softmax shift-invariance + late normalization).

Sharding: data-parallel over batch, 512 -> 8 cores x 64.

Per-core layout (P = 128 partitions):
  - einsum operands live in a "spread" layout: r = 16k + m (k in 0..72,
    m in 0..16); partition p(m,i) = (m%4)*32 + (m//4)*8 + i.  The 4
    MMs of 4 consecutive m hit 4 distinct 32-row groups of the PE
    array and run concurrently (tile_position row packing).
  - u_hat is resident in SBUF as bf16 [128 = (rq, b32); 288, 16(o), 10(c)]
    with partition = rq*32 + b_local (rq = r quartile), per 32-batch
    tile (2 tiles per core).  Free order is (rs, o, c) -- c innermost --
    so every elementwise product keeps a packed 2-byte last dim and runs
    in the DVE 2x perf mode, including broadcasts of V (over rs) and of
    E=exp(z) (over o).
  - iter-0 s (uniform coef 0.1) is a clean K=128 chunked matmul.
  - routing iters 1..2: products u_hat*V / u_hat*E split DVE/GPSIMD;
    all o/rs reductions are pairwise tree-adds in bf16 (2x mode) instead
    of TensorReduce (which has no DVE perf modes).  Cross-partition-group
    reductions (sum over the 4 rq groups) and the V broadcast to the
    (rq,b) layout are done with tiny constant matmuls (REP/REPT).
"""

import sys

sys.path.insert(0, "/opt/trn_rl_repo")

import functools
from contextlib import ExitStack

import numpy as np

NCORES = 8
B = 64          # batch per core
BT = 32         # batch tile
R = 1152
C = 10
I = 8
O = 16
CO = C * O      # 160
NK = 72         # r-chunks of 16
RQ_K = 18       # k's per r-quartile (288 r's)
RL = 288        # r_loc per quartile
ZCH = 32        # r_locs per routing chunk
NCH = RL // ZCH  # 9
POOL_PROD = 10   # of the 18 products per tile-iter, how many go to Pool
USE_TPOS = True     # pass explicit tile_position on einsum MMs


def _wslice(w):
    return slice(w * 32, (w + 1) * 32)


def build_bass(phase: str = "full"):
    import concourse.bass as bass
    import concourse.tile as tile
    from concourse import bacc, mybir
    from concourse.masks import make_identity

    f32 = mybir.dt.float32
    bf16 = mybir.dt.bfloat16
    AX = mybir.AxisListType
    OP = mybir.AluOpType
    AF = mybir.ActivationFunctionType

    nc = bacc.Bacc(
        "TRN2",
        target_bir_lowering=False,
        debug=False,
        enable_asserts=False,
        num_devices=NCORES,
    )
    # u and W arrive pre-arranged (host side) in the SBUF spread layouts:
    # ut[p, k, b] = u[b, 16k+m(p), i(p)], wsb[p, k, o, c] = W[16k+m(p), c,
    # i(p), o] with p(m,i) = (m%4)*32 + (m//4)*8 + i, both bf16 -> prep is
    # two big contiguous DMAs instead of 160 strided gathers + casts.
    u_d = nc.dram_tensor("ut", [128, NK, B], bf16, kind="ExternalInput").ap()
    w_d = nc.dram_tensor("wsb", [128, NK, O, C], bf16, kind="ExternalInput").ap()
    v_d = nc.dram_tensor("v", [B, C, O], f32, kind="ExternalOutput").ap()
    m_d = nc.dram_tensor("msk", [128, 4], f32, kind="ExternalInput").ap()

    with tile.TileContext(nc) as tc, ExitStack() as ctx:
        # ---------------- persistent pools ----------------
        consts = ctx.enter_context(tc.tile_pool(name="consts", bufs=1))
        persist = ctx.enter_context(tc.tile_pool(name="persist", bufs=1))

        # uTz[j] holds u^T (partition p = (r%16)*8 + i) with only the
        # rows of m % 4 == j kept, zeros elsewhere -> a K=32 matmul on a
        # 32-aligned window isolates one r.
        uTz0 = persist.tile([128, NK, B], bf16)
        uTz1 = persist.tile([128, NK, B], bf16)
        uTz2 = persist.tile([128, NK, B], bf16)
        uTz3 = persist.tile([128, NK, B], bf16)
        uTz = [uTz0, uTz1, uTz2, uTz3]
        W_sb = persist.tile([128, NK, O, C], bf16)  # 23 KB/part, (o,c) order

        def ecopy(which, out_ap, in_ap):
            if which == 0:
                nc.vector.tensor_copy(out_ap, in_ap)
            else:
                nc.scalar.copy(out_ap, in_ap)

        id64 = consts.tile([64, 64], bf16)
        make_identity(nc, id64)
        id32 = consts.tile([32, 32], f32)
        make_identity(nc, id32)

        # REP[b, (q, b')] = 1.0 iff b == b'   (bf16, [32, 4, 32])
        REPf = consts.tile([32, 4, 32], f32)
        nc.gpsimd.memset(REPf[:], 0.0)
        nc.gpsimd.affine_select(
            out=REPf[:],
            in_=REPf[:],
            compare_op=OP.not_equal,
            fill=1.0,
            base=0,
            pattern=[[0, 4], [-1, 32]],
            channel_multiplier=1,
        )
        REPT = consts.tile([128, 32], bf16)

        # ---------------- routing-side pools needed inside prep (iter0) --
        rt = ctx.enter_context(tc.tile_pool(name="rt", bufs=1))
        sm = ctx.enter_context(tc.tile_pool(name="sm", bufs=1))
        logits = rt.tile([128, RL, C], f32)          # 11.5 KB
        E_slim = rt.tile([128, RL, C], bf16)         # 5.6 KB
        den = rt.tile([128, RL], f32)
        V_rep = rt.tile([128, O, C], bf16)
        s_acc = rt.tile([128, O, C], f32)
        s_bf = rt.tile([128, CO], bf16)
        v0 = rt.tile([64, O, C], f32)
        Vcb0 = rt.tile([32, O, C], f32)
        Vcb1 = rt.tile([32, O, C], f32)
        Vcb = [Vcb0, Vcb1]

        def squash(p, s_ap, out_ap, pool):
            # out = |s| / (1 + |s|^2) * s   per (partition, c); s is [p, O, C]
            sq = pool.tile([p, O, C], f32, tag="sqt")
            nc.vector.tensor_mul(sq[:], s_ap, s_ap)
            ssum = pool.tile([p, C], f32, tag="sst")
            nc.vector.tensor_reduce(
                ssum[:], sq[:].rearrange("p o c -> p c o"), axis=AX.X, op=OP.add
            )
            norm = pool.tile([p, C], f32, tag="snt")
            nc.scalar.sqrt(norm[:], ssum[:])
            onep = pool.tile([p, C], f32, tag="sot")
            nc.scalar.add(onep[:], ssum[:], 1.0)
            rec = pool.tile([p, C], f32, tag="srt")
            nc.vector.reciprocal(rec[:], onep[:])
            fac = pool.tile([p, C], f32, tag="sft")
            nc.vector.tensor_mul(fac[:], norm[:], rec[:])
            nc.vector.tensor_mul(
                out_ap,
                s_ap,
                fac[:].unsqueeze(1).broadcast_to((p, O, C)),
            )

        def iter0(s0ps):
            s_all = rt.tile([64, O, C], f32)
            nc.scalar.mul(
                s_all[:], s0ps[:].rearrange("p (o c) -> p o c", o=O), 0.1
            )
            squash(64, s_all[:], v0[:], sm)
            nc.vector.tensor_copy(Vcb[0][:], v0[0:32, :, :])
            nc.sync.dma_start(out=Vcb[1][:], in_=v0[32:64, :, :])

        # ---------------- prep phase ----------------
        with ExitStack() as prep:
            pp = prep.enter_context(tc.tile_pool(name="prep", bufs=1))
            wch = prep.enter_context(tc.tile_pool(name="wch", bufs=1))
            ppsum = prep.enter_context(
                tc.tile_pool(name="ppsum", bufs=2, space="PSUM")
            )
            s0_pool = prep.enter_context(
                tc.tile_pool(name="s0psp", bufs=1, space="PSUM")
            )
            s0ps = s0_pool.tile([64, CO], f32)

            # REPT = REP^T via PE
            rps = ppsum.tile([128, 32], f32)
            nc.tensor.transpose(
                rps[:], REPf[:].rearrange("b q c -> b (q c)"), id32[:]
            )
            nc.vector.tensor_copy(REPT[:], rps[:])

            # u^T and W: single contiguous DMAs into the spread layouts
            uT_full = pp.tile([128, NK, B], bf16)
            nc.sync.dma_start(
                out=uT_full[:].rearrange("p k b -> p (k b)"),
                in_=u_d.rearrange("p k b -> p (k b)"),
            )
            nc.gpsimd.dma_start(
                out=W_sb[:].rearrange("p k o c -> p (k o c)"),
                in_=w_d.rearrange("p k o c -> p (k o c)"),
            )
            msk = pp.tile([128, 4], f32)
            nc.sync.dma_start(out=msk[:], in_=m_d)
            for j in range(4):
                nc.vector.tensor_scalar_mul(
                    uTz[j][:].rearrange("p k b -> p (k b)"),
                    uT_full[:].rearrange("p k b -> p (k b)"),
                    msk[:, j : j + 1],
                )

            # iter-0 s matmul chain while the full u^T is still alive:
            # s0 = sum_k uT_full[:,k,:].T @ W_sb[:,k,:]  (all 64 b at once)
            for k in range(NK):
                nc.tensor.matmul(
                    s0ps[:],
                    uT_full[:, k, :],
                    W_sb[:, k, :, :],
                    start=(k == 0),
                    stop=(k == NK - 1),
                )
            if phase != "prep":
                iter0(s0ps)

        # ---------------- main pools ----------------
        big = ctx.enter_context(tc.tile_pool(name="big", bufs=1))
        scratch = ctx.enter_context(tc.tile_pool(name="scratch", bufs=2))
        u_hat = big.tile([128, RL, O, C], bf16)      # 92 KB/part, (rs,o,c)

        # ---------------- routing z chunk (shared) ----------------
        def z_chunk(rc):
            uh = u_hat[:, rc * ZCH : (rc + 1) * ZCH, :, :].rearrange(
                "p r o c -> p r (o c)"
            )
            pr = scratch.tile([128, ZCH, CO], bf16, tag="pr")
            vb = (
                V_rep[:].rearrange("p o c -> p (o c)")
                .unsqueeze(1).broadcast_to((128, ZCH, CO))
            )
            peng = nc.gpsimd if rc % 4 == 0 else nc.vector
            peng.tensor_mul(pr[:], uh, vb)
            nc.vector.tensor_add(pr[:, :, 0:80], pr[:, :, 0:80], pr[:, :, 80:160])
            nc.vector.tensor_add(pr[:, :, 0:40], pr[:, :, 0:40], pr[:, :, 40:80])
            nc.vector.tensor_add(pr[:, :, 0:20], pr[:, :, 0:20], pr[:, :, 20:40])
            nc.vector.tensor_add(
                logits[:, rc * ZCH : (rc + 1) * ZCH, :],
                pr[:, :, 0:10], pr[:, :, 10:20],
            )

        # ---------------- einsum: u_hat per batch tile ----------------
        def einsum_tile(bt, mm_psum, fuse_z=False):
            if fuse_z:
                # V_rep for the first routing iter, using the einsum's own
                # PSUM tag so no extra bank is needed; z chunks are issued
                # as their u_hat windows complete and run in einsum's DVE
                # slack.
                vps = mm_psum.tile([128, 4, 512], f32, tag="pe")
                nc.tensor.matmul(
                    vps[:, 0, 0:CO],
                    REPf[:].rearrange("b q c -> b (q c)"),
                    Vcb[bt][:].rearrange("p o c -> p (o c)"),
                    start=True,
                    stop=True,
                )
                nc.vector.tensor_copy(
                    V_rep[:].rearrange("p o c -> p (o c)"), vps[:, 0, 0:CO]
                )
            _einsum_tile(bt, mm_psum, fuse_z)

        def _einsum_tile(bt, mm_psum, fuse_z):
            # storage index within a 16-block: rs = j*4 + w (the 4
            # w-concurrent MMs land on consecutive r slots).  Any r
            # permutation is fine: routing is symmetric in r.
            for kl in range(RQ_K):
                for j in range(4):
                    pe_ps = mm_psum.tile([128, 4, 512], f32, tag="pe")
                    for rq in range(4):
                        k = rq * RQ_K + kl
                        for w in range(4):
                            nc.tensor.matmul(
                                pe_ps[rq * 32 : (rq + 1) * 32, w, 0:CO],
                                uTz[j][_wslice(w), k, bt * BT : (bt + 1) * BT],
                                W_sb[_wslice(w), k, :, :],
                                start=True,
                                stop=True,
                                tile_position=(w * 32, rq * 32),
                            )
                    rs0 = 16 * kl + j * 4
                    dst = u_hat[:, rs0 : rs0 + 4, :, :].rearrange(
                        "p r o c -> p r (o c)"
                    )
                    ecopy(j % 2, dst, pe_ps[:, :, 0:CO])
                if fuse_z and kl % 2 == 1:
                    z_chunk((kl - 1) // 2)

        # ---------------- routing iteration ----------------
        def routing_iter(bt, t, it_psum, skip_z=False):
            # z-pass: logits = sum_o u_hat * V (see z_chunk).  When fused
            # into the einsum (skip_z), V_rep and all logits chunks were
            # already issued there.
            if not skip_z:
                vps = it_psum.tile([128, CO], f32, tag="vrep")
                nc.tensor.matmul(
                    vps[:],
                    REPf[:].rearrange("b q c -> b (q c)"),
                    Vcb[bt][:].rearrange("p o c -> p (o c)"),
                    start=True,
                    stop=True,
                )
                nc.vector.tensor_copy(
                    V_rep[:].rearrange("p o c -> p (o c)"), vps[:]
                )
                for rc in range(NCH):
                    z_chunk(rc)
            # softmax over c: E = exp(logits) then normalize by the per-r
            # class sum.  No max-subtraction: |logits| <~ 60 is inside fp32
            # exp range.
            nc.scalar.activation(
                logits[:].rearrange("p r c -> p (r c)"),
                logits[:].rearrange("p r c -> p (r c)"),
                AF.Exp,
            )
            nc.vector.tensor_reduce(den[:], logits[:], axis=AX.X, op=OP.add)
            nc.vector.reciprocal(den[:], den[:])
            nc.vector.tensor_mul(
                E_slim[:],
                logits[:],
                den[:].unsqueeze(2).broadcast_to((128, RL, C)),
            )
            # s-pass: s_acc = sum_rloc E * u_hat (E broadcast over o; packed
            # c last -> 2x).  rs reduced by tree adds (halving the rs dim).
            nc.gpsimd.memset(s_acc[:], 0.0)
            for rc in range(NCH):
                uh4 = u_hat[:, rc * ZCH : (rc + 1) * ZCH, :, :]
                pr2 = scratch.tile([128, ZCH, O, C], bf16, tag="pr")
                peng = nc.gpsimd if rc % 3 == 1 else nc.vector
                # E broadcast over o is inherently 4D -> TensorTensor (2x)
                peng.tensor_mul(
                    pr2[:],
                    uh4,
                    E_slim[:, rc * ZCH : (rc + 1) * ZCH, :]
                    .unsqueeze(2)
                    .broadcast_to((128, ZCH, O, C)),
                )
                # in-place pairwise tree over rs (outer dim, bf16 2x)
                p3 = pr2[:].rearrange("p r o c -> p r (o c)")
                nc.vector.tensor_add(p3[:, 0:16], p3[:, 0:16], p3[:, 16:32])
                nc.vector.tensor_add(p3[:, 0:8], p3[:, 0:8], p3[:, 8:16])
                nc.vector.tensor_add(p3[:, 0:4], p3[:, 0:4], p3[:, 4:8])
                nc.vector.tensor_add(p3[:, 0:2], p3[:, 0:2], p3[:, 2:4])
                nc.vector.tensor_add(p3[:, 0], p3[:, 0], p3[:, 1])
                nc.vector.tensor_add(
                    s_acc[:].rearrange("p o c -> p (o c)"),
                    s_acc[:].rearrange("p o c -> p (o c)"), p3[:, 0]
                )
            # combine the 4 rq groups via one REPT matmul
            nc.vector.tensor_copy(
                s_bf[:], s_acc[:].rearrange("p o c -> p (o c)")
            )
            scps = it_psum.tile([32, CO], f32, tag="comb")
            nc.tensor.matmul(
                scps[:], REPT[:], s_bf[:], start=True, stop=True
            )
            s_bt = sm.tile([32, O, C], f32, tag="sbt")
            nc.vector.tensor_copy(
                s_bt[:].rearrange("p o c -> p (o c)"), scps[:]
            )
            if t == 1:
                v_t = sm.tile([32, O, C], f32, tag="vt")
                squash(32, s_bt[:], v_t[:], sm)
                nc.vector.tensor_add(Vcb[bt][:], Vcb[bt][:], v_t[:])
            else:
                # write v in (c, o) order so the output DMA is contiguous
                v_out = sm.tile([32, C, O], f32, tag="vout")
                squash(32, s_bt[:], v_out[:].rearrange("p c o -> p o c"), sm)
                nc.sync.dma_start(
                    out=v_d[bt * BT : (bt + 1) * BT, :, :], in_=v_out[:]
                )

        if phase == "prep":
            pass
        elif phase == "einsum":
            with ExitStack() as es:
                mm_psum = es.enter_context(
                    tc.tile_pool(name="mmps0", bufs=2, space="PSUM")
                )
                einsum_tile(0, mm_psum)
        else:
            for bt in range(2):
                with ExitStack() as es:
                    mm_psum = es.enter_context(
                        tc.tile_pool(name=f"mmps{bt}", bufs=2, space="PSUM")
                    )
                    einsum_tile(bt, mm_psum, fuse_z=True)
                with ExitStack() as es:
                    it_psum = es.enter_context(
                        tc.tile_pool(name=f"itps{bt}", bufs=2, space="PSUM")
                    )
                    routing_iter(bt, 1, it_psum, skip_z=True)
                    routing_iter(bt, 2, it_psum)

    nc.compile()
    return nc


@functools.cache
def _get_nc():
    return build_bass()


def make_mask() -> np.ndarray:
    p = np.arange(128)
    j = (p // I) % 4
    return (j[:, None] == np.arange(4)[None, :]).astype(np.float32)


def kernel(u: np.ndarray, W: np.ndarray) -> np.ndarray:
    import ml_dtypes
    from concourse import bass_utils

    nc = _get_nc()
    bf = ml_dtypes.bfloat16
    W4 = np.ascontiguousarray(W.reshape(R, C, I, O)).astype(np.float32)
    # spread layout: partition p(m, i) = m*8 + i (transpose column order)
    p = np.arange(128)
    m = p // 8
    i = p % 8
    r_idx = 16 * np.arange(NK)[None, :] + m[:, None]          # [128, NK]
    # wsb[p, k, o, c] = W[16k+m(p), c, i(p), o]
    wsb = np.ascontiguousarray(
        W4[r_idx, :, i[:, None], :].transpose(0, 1, 3, 2)
    ).astype(bf)
    msk = make_mask()
    in_maps = []
    for core in range(NCORES):
        uc = np.asarray(u[core * B : (core + 1) * B], dtype=np.float32)
        # ut[p, k, b] = u[b, 16k+m(p), i(p)]
        ut = np.ascontiguousarray(
            uc[:, r_idx, i[:, None]].transpose(1, 2, 0)
        ).astype(bf)
        in_maps.append({"ut": ut, "wsb": wsb, "msk": msk})
    res = bass_utils.run_bass_kernel_spmd(
        nc, in_maps, core_ids=list(range(NCORES))
    )
    return np.concatenate([r["v"] for r in res.results], axis=0)
